# revision 5
# baseline (speedup 1.0000x reference)
"""Bass/Trainium2 LSTM encoder kernel.

Problem: nn_Encoder (LSTM): input [B=4096, T=512, IN=22], hidden H=64,
torch gate order i,f,g,o. Output: hidden states [B, T, H].

Sharding: data-parallel over batch across 8 NeuronCores (512 batch rows per
core, split into two software-pipelined streams of 256). Weights replicated.
The T=512 recurrence runs sequentially per core.

Per-core structure (feature-on-partition, batch in the free dim), bf16
datapath (PSUM accumulation stays fp32):
  - x host-transposed to xT [T, 23, B] bf16; row 22 is ones, so the bias
    rides the x-matmul (K=23).
  - Stationary S1/S2 [128, 128] bf16: rows 0:22 W_ihT gate-chunk, row 22
    combined bias, rows 64:128 W_hhT gate-chunk. Per stream-step, two
    row-tiled matmuls per chunk (x-part at array rows 0:23, h-part at rows
    64:128) accumulate one psum bank [128, 512] (chunk1 cols 0:256,
    chunk2 256:512).
  - Uniform-tanh trick: sigmoid-gate rows (i, f, o) of W and bias are
    pre-scaled 0.5 on the host (sigmoid(z) = 0.5 tanh(0.5 z) + 0.5), so ONE
    tanh activation over the whole psum produces all gates as bf16:
    G = [ti;tf|tg;to].
  - Scaled state C = 2c and history h' = 2h (host halves the output); all
    elementwise ops are bf16 scalar_tensor_tensor on DVE (2-byte dtype
    engages the DVE fast modes):
      u' = (ti+1)*tg
      W  = (tf+1)*C
      C  = 0.5 W + u'       (C persistent per stream, memset once)
      tc = tanh(0.5 C)      activation with input scale (ACT)
      h' = (to+1)*tc
  - Two batch streams of 256 are interleaved so the per-step dependency
    chain of one stream overlaps the other stream's engine work.
  - h' written into an SBUF history strip (base partition 64, aligned with
    the W_hh array rows), DMA'd out every TC steps as hs [T, H, B] bf16; the
    host transposes back to [B, T, H] and multiplies by 0.5.
Walrus in this container accepts at most ONE semaphore wait per instruction;
_split_waits post-processes Tile's output to satisfy that.
"""

import numpy as np
import ml_dtypes

import bass_rust
import concourse.bass as bass
import concourse.mybir as mybir
import concourse.tile as tile
import concourse.bass_utils as bass_utils

N_CORES = 8
B_FULL, T, IN, H = 4096, 512, 22, 64
B = B_FULL // N_CORES          # batch per core
BS = B // 2                    # batch per stream
KX = IN + 1                    # x rows + ones row
TC = 16                        # timesteps per DMA chunk
F32 = mybir.dt.float32
BF16 = mybir.dt.bfloat16

_cache = {}


def _split_waits(nc, max_waits=1):
    """walrus here allows one sem-wait per instruction; split extras into
    preceding same-engine NOPs."""
    for f in nc.m.functions:
        for bb in f.blocks:
            insts = bb.instructions
            changed = False
            out = []
            for inst in insts:
                si = inst.sync_info
                if si is not None and si.on_wait and len(si.on_wait) > max_waits:
                    waits = list(si.on_wait)
                    head, rest = waits[:-max_waits], waits[-max_waits:]
                    for i in range(0, len(head), max_waits):
                        nop = mybir.InstNoOp(name=nc.get_next_instruction_name())
                        nop.engine = inst.engine
                        nop.sync_info = bass_rust.SyncInfo(
                            on_wait=head[i:i + max_waits], on_update=[])
                        out.append(nop)
                    inst.sync_info = bass_rust.SyncInfo(
                        on_wait=rest, on_update=list(si.on_update))
                    changed = True
                out.append(inst)
            if changed:
                cur = bb.instructions
                del cur[:]
                cur.extend(out)


def _build():
    if "nc" in _cache:
        return _cache["nc"]

    nc = bass.Bass("TRN2", target_bir_lowering=False, debug=False,
                   enable_asserts=False, num_devices=1)

    xT_d = nc.dram_tensor("xT", [T, KX, B], BF16, kind="ExternalInput").ap()
    s1_d = nc.dram_tensor("S1", [128, 128], BF16, kind="ExternalInput").ap()
    s2_d = nc.dram_tensor("S2", [128, 128], BF16, kind="ExternalInput").ap()
    hs_d = nc.dram_tensor("hs", [T, H, B], BF16, kind="ExternalOutput").ap()

    TANH = mybir.ActivationFunctionType.Tanh
    ADD = mybir.AluOpType.add
    MUL = mybir.AluOpType.mult

    n_chunks = T // TC

    with tile.TileContext(nc) as tc:
        with (
            tc.tile_pool(name="const", bufs=1) as cpool,
            tc.tile_pool(name="xin", bufs=3) as xpool,
            tc.tile_pool(name="hh", bufs=2) as hpool,
            tc.tile_pool(name="gates", bufs=6) as gpool,
            tc.tile_pool(name="tmp", bufs=8) as tpool,
            tc.tile_pool(name="cst", bufs=1) as stpool,
            tc.tile_pool(name="ps", bufs=4, space="PSUM") as pspool,
        ):
            s1 = cpool.tile([128, 128], BF16, tag="s1")
            s2 = cpool.tile([128, 128], BF16, tag="s2")
            nc.sync.dma_start(s1[:], s1_d[:])
            nc.sync.dma_start(s2[:], s2_d[:])

            # persistent scaled cell state per stream (C = 2c), bf16.
            # Lives on partitions 64:128 so stst inputs pairing it with
            # g[64:128] share a base partition (walrus NCC_IBIR297).
            cst = [stpool.tile([128, BS], BF16, tag=f"c{s}", name=f"cstate{s}")
                   for s in (0, 1)]
            cstate = [t[64:128, :] for t in cst]
            for s in (0, 1):
                nc.vector.memset(cstate[s], 0.0)

            h_prev = [None, None]   # AP of h_{t-1} per stream
            for ci in range(n_chunks):
                xch = xpool.tile([KX, TC * B], BF16, tag="x")
                nc.sync.dma_start(
                    xch[:].rearrange("k (t b) -> k t b", t=TC),
                    xT_d[ci * TC:(ci + 1) * TC].rearrange("t k b -> k t b"),
                )
                hh = hpool.tile([128, TC * B], BF16, tag="h")
                for j in range(TC):
                    for s in (0, 1):
                        off = j * B + s * BS
                        rx = xch[:, off:off + BS]
                        ps = pspool.tile([128, 2 * BS], F32, tag="ps")
                        first = h_prev[s] is None
                        nc.tensor.matmul(ps[:, 0:BS], s1[0:KX, :], rx,
                                         start=True, stop=first,
                                         tile_position=(0, 0))
                        if not first:
                            nc.tensor.matmul(ps[:, 0:BS], s1[64:128, :],
                                             h_prev[s], start=False, stop=True,
                                             tile_position=(64, 0))
                        nc.tensor.matmul(ps[:, BS:2 * BS], s2[0:KX, :], rx,
                                         start=True, stop=first,
                                         tile_position=(0, 0))
                        if not first:
                            nc.tensor.matmul(ps[:, BS:2 * BS], s2[64:128, :],
                                             h_prev[s], start=False, stop=True,
                                             tile_position=(64, 0))

                        # one uniform tanh over both gate chunks -> bf16
                        g = gpool.tile([128, 2 * BS], BF16, tag="g")
                        nc.scalar.activation(g[:], ps[:], TANH)
                        # G layout: cols 0:BS = [ti; tf], cols BS:2BS = [tg; to]
                        # u' = (ti+1)*tg ; W = (tf+1)*C ; C = 0.5W + u' ;
                        # tc = tanh(0.5*C) ; h' = (to+1)*tc  (all bf16 on DVE)
                        up = tpool.tile([64, BS], BF16, tag="up")
                        nc.vector.scalar_tensor_tensor(
                            up[:], g[0:H, 0:BS], 1.0,
                            g[0:H, BS:2 * BS], op0=ADD, op1=MUL)
                        wt = tpool.tile([64, BS], BF16, tag="wt")
                        nc.vector.scalar_tensor_tensor(
                            wt[:], g[H:128, 0:BS], 1.0, cstate[s],
                            op0=ADD, op1=MUL)
                        nc.vector.scalar_tensor_tensor(
                            cstate[s], wt[:], 0.5, up[:],
                            op0=MUL, op1=ADD)
                        tcb = tpool.tile([128, BS], BF16, tag="tc")
                        nc.scalar.activation(tcb[64:128, :], cstate[s], TANH,
                                             scale=0.5)
                        h_out = hh[64:128, off:off + BS]
                        nc.vector.scalar_tensor_tensor(
                            h_out, g[H:128, BS:2 * BS], 1.0, tcb[64:128, :],
                            op0=ADD, op1=MUL)
                        h_prev[s] = h_out
                nc.sync.dma_start(
                    hs_d[ci * TC:(ci + 1) * TC].rearrange("t h b -> h t b"),
                    hh[64:128, :].rearrange("h (t b) -> h t b", t=TC),
                )

    _split_waits(nc, max_waits=1)
    _cache["nc"] = nc
    return nc


def _prep_core_inputs(input_data, W_ih, W_hh, b_ih, b_hh):
    bias = (b_ih + b_hh).astype(np.float32)           # [256]
    W_ihT = W_ih.astype(np.float32).T.copy()          # [22, 256]
    W_hhT = W_hh.astype(np.float32).T.copy()          # [64, 256]
    # scale sigmoid-gate rows (i: 0:64, f: 64:128, o: 192:256) by 0.5 for
    # the uniform-tanh trick; g rows (128:192) stay unscaled
    scale = np.ones(256, np.float32) * 0.5
    scale[128:192] = 1.0
    W_ihT *= scale
    bias *= scale
    # W_hh consumes h' = 2h from the history strip -> extra 0.5
    W_hhT *= scale * 0.5

    def stationary(lo, hi):
        s = np.zeros((128, 128), np.float32)
        s[0:IN, :] = W_ihT[:, lo:hi]
        s[IN, :] = bias[lo:hi]
        s[64:128, :] = W_hhT[:, lo:hi]
        return s.astype(ml_dtypes.bfloat16)

    s1 = stationary(0, 128)
    s2 = stationary(128, 256)

    x8 = input_data.reshape(N_CORES, B, T, IN)
    in_maps = []
    for c in range(N_CORES):
        xT = np.empty((T, KX, B), ml_dtypes.bfloat16)
        xT[:, 0:IN, :] = x8[c].transpose(1, 2, 0).astype(ml_dtypes.bfloat16)
        xT[:, IN, :] = 1.0
        in_maps.append({"xT": np.ascontiguousarray(xT), "S1": s1, "S2": s2})
    return in_maps


def kernel(input_data, W_ih, W_hh, b_ih, b_hh):
    input_data = np.asarray(input_data, np.float32)
    W_ih = np.asarray(W_ih, np.float32)
    W_hh = np.asarray(W_hh, np.float32)
    b_ih = np.asarray(b_ih, np.float32)
    b_hh = np.asarray(b_hh, np.float32)

    nc = _build()
    in_maps = _prep_core_inputs(input_data, W_ih, W_hh, b_ih, b_hh)
    res = bass_utils.run_bass_kernel_spmd(nc, in_maps, core_ids=list(range(N_CORES)))
    _cache["last_results"] = res

    out = np.empty((B_FULL, T, H), np.float32)
    for c in range(N_CORES):
        hs = res.results[c]["hs"].astype(np.float32)  # [T, H, B] (holds 2h)
        out[c * B:(c + 1) * B] = hs.transpose(2, 0, 1)
    out *= 0.5
    return out


# revision 18
# speedup vs baseline: 1.2543x; 1.2543x over previous
"""Bass/Trainium2 LSTM encoder kernel.

Problem: nn_Encoder (LSTM): input [B=4096, T=512, IN=22], hidden H=64,
torch gate order i,f,g,o. Output: hidden states [B, T, H].

Sharding: data-parallel over batch across 8 NeuronCores (512 batch rows per
core, split into two software-pipelined streams of 256). Weights replicated.
The T=512 recurrence runs sequentially per core.

Per-core structure (feature-on-partition, batch in the free dim), bf16
datapath (PSUM accumulation stays fp32):
  - x host-transposed to xT [T, 23, B] bf16; row 22 is ones, so the bias
    rides the x-matmul (K=23).
  - Stationary S1/S2 [128, 128] bf16: rows 0:22 W_ihT gate-chunk, row 22
    combined bias, rows 64:128 W_hhT gate-chunk. Per stream-step, two
    row-tiled matmuls per chunk (x-part at array rows 0:23, h-part at rows
    64:128) accumulate one psum bank [128, 512] (chunk1 cols 0:256,
    chunk2 256:512).
  - Uniform-tanh trick: sigmoid-gate rows (i, f, o) of W and bias are
    pre-scaled 0.5 on the host (sigmoid(z) = 0.5 tanh(0.5 z) + 0.5), so ONE
    tanh activation over the whole psum produces all gates as bf16:
    G = [ti;tf|tg;to].
  - Scaled state C = 2c and history h' = 2h (host halves the output); all
    elementwise ops are bf16 scalar_tensor_tensor on DVE (2-byte dtype
    engages the DVE fast modes):
      u' = (ti+1)*tg
      W  = (tf+1)*C
      C  = 0.5 W + u'       (C persistent per stream, memset once)
      tc = tanh(0.5 C)      activation with input scale (ACT)
      h' = (to+1)*tc
  - Two batch streams of 256 are interleaved so the per-step dependency
    chain of one stream overlaps the other stream's engine work.
  - h' written into an SBUF history strip (base partition 64, aligned with
    the W_hh array rows), DMA'd out every TC steps as hs [T, H, B] bf16; the
    host transposes back to [B, T, H] and multiplies by 0.5.
Walrus in this container accepts at most ONE semaphore wait per instruction;
_split_waits post-processes Tile's output to satisfy that.
"""

import numpy as np
import ml_dtypes

import bass_rust
import concourse.bass as bass
import concourse.mybir as mybir
import concourse.tile as tile
import concourse.bass_utils as bass_utils

N_CORES = 8
B_FULL, T, IN, H = 4096, 512, 22, 64
B = B_FULL // N_CORES          # batch per core
NS = 4                         # software-pipelined streams
BS = B // NS                   # batch per stream
LAG = 2                        # quarter-slots between a head and its tail
KX = IN + 1                    # x rows + ones row
TC = 16                        # timesteps per DMA chunk
F32 = mybir.dt.float32
BF16 = mybir.dt.bfloat16

_cache = {}
ROLES = {}      # instruction name -> (role, stream, step); for trace analysis


def _tag(inst, role, s, j):
    try:
        ROLES[inst.ins.name] = (role, s, j)
    except Exception:
        pass
    return inst


def _split_waits(nc, max_waits=1):
    """walrus here allows one sem-wait per instruction; split extras into
    preceding same-engine NOPs."""
    for f in nc.m.functions:
        for bb in f.blocks:
            insts = bb.instructions
            changed = False
            out = []
            for inst in insts:
                si = inst.sync_info
                if si is not None and si.on_wait and len(si.on_wait) > max_waits:
                    waits = list(si.on_wait)
                    head, rest = waits[:-max_waits], waits[-max_waits:]
                    for i in range(0, len(head), max_waits):
                        nop = mybir.InstNoOp(name=nc.get_next_instruction_name())
                        nop.engine = inst.engine
                        nop.sync_info = bass_rust.SyncInfo(
                            on_wait=head[i:i + max_waits], on_update=[])
                        out.append(nop)
                    inst.sync_info = bass_rust.SyncInfo(
                        on_wait=rest, on_update=list(si.on_update))
                    changed = True
                out.append(inst)
            if changed:
                cur = bb.instructions
                del cur[:]
                cur.extend(out)


def _build():
    if "nc" in _cache:
        return _cache["nc"]

    nc = bass.Bass("TRN2", target_bir_lowering=False, debug=False,
                   enable_asserts=False, num_devices=1)

    xT_d = nc.dram_tensor("xT", [T, KX, B], BF16, kind="ExternalInput").ap()
    s1_d = nc.dram_tensor("S1", [128, 128], BF16, kind="ExternalInput").ap()
    s2_d = nc.dram_tensor("S2", [128, 128], BF16, kind="ExternalInput").ap()
    p_d = nc.dram_tensor("P", [128, 128], BF16, kind="ExternalInput").ap()
    hs_d = nc.dram_tensor("hs", [T, H, B], BF16, kind="ExternalOutput").ap()

    TANH = mybir.ActivationFunctionType.Tanh
    ADD = mybir.AluOpType.add
    MUL = mybir.AluOpType.mult

    n_chunks = T // TC

    with tile.TileContext(nc) as tc:
        with (
            tc.tile_pool(name="const", bufs=1) as cpool,
            tc.tile_pool(name="xin", bufs=3) as xpool,
            tc.tile_pool(name="hh", bufs=2) as hpool,
            tc.tile_pool(name="gates", bufs=8) as gpool,
            tc.tile_pool(name="tmp", bufs=8) as tpool,
            tc.tile_pool(name="ps", bufs=4, space="PSUM") as pspool,
            tc.tile_pool(name="cs", bufs=4, space="PSUM") as cspool,
        ):
            s1 = cpool.tile([128, 128], BF16, tag="s1")
            s2 = cpool.tile([128, 128], BF16, tag="s2")
            pmat = cpool.tile([128, 128], BF16, tag="pmat")
            nc.sync.dma_start(s1[:], s1_d[:])
            nc.sync.dma_start(s2[:], s2_d[:])
            nc.sync.dma_start(pmat[:], p_d[:])

            # Software-pipelined schedule over (stream, step) slots.
            # At slot k (stream s = k % NS, step j = k // NS) we emit, in
            # program order per engine, the HEAD of (s, j) and the TAIL of
            # the slot-(k-LAG) stream-step (sp, jp):
            #   PE : 4 gate matmuls (s,j) ; cps (sp,jp)  [C' = 0.5W + u']
            #   ACT: tanh gates (s,j)     ; tanh-c (sp,jp)
            #   DVE: u',W (s,j)           ; h' (sp,jp)
            # The LAG matches the g->wu->cps latency so no engine ever
            # blocks in-order on a cross-engine round trip: every tail op's
            # inputs are ready by the time its engine reaches it.
            h_prev = [None] * NS     # h'(s, j-1) AP
            c_prev = [None] * NS     # psum C AP per stream (None -> zeros)
            pend_W = []              # deferred W-stst emitters
            xchs = {}
            hhs = {}

            def emit_slot(s, j, pend):
                ci = j // TC
                if ci not in xchs:
                    xch = xpool.tile([KX, TC * B], BF16, tag="x",
                                     name=f"xch{ci}")
                    nc.sync.dma_start(
                        xch[:].rearrange("k (t b) -> k t b", t=TC),
                        xT_d[ci * TC:(ci + 1) * TC].rearrange("t k b -> k t b"),
                    )
                    xchs[ci] = xch
                    hhs[ci] = hpool.tile([128, TC * B], BF16, tag="h",
                                         name=f"hh{ci}")
                xch, hh = xchs[ci], hhs[ci]
                off = (j - ci * TC) * B + s * BS
                rx = xch[:, off:off + BS]
                ps = pspool.tile([128, 2 * BS], F32, tag="ps", name=f"ps{s}{j}")
                first = h_prev[s] is None
                # --- PE: pending tail's cps first (input ready since the
                # previous slot; must not queue behind the h-matmul hazard) ---
                if pend is not None:
                    sp, jp, wup = pend[0], pend[1], pend[2]
                    cps = cspool.tile([128, BS], F32, tag="cps",
                                      name=f"cps{sp}{jp}")
                    _tag(nc.tensor.matmul(cps[:], pmat[:], wup[:], start=True,
                                     stop=True, tile_position=(0, 0)),
                         "cps", sp, jp)
                    c_prev[sp] = cps[64:128, :]
                # --- PE: gate matmuls. Each chunk's start..stop accumulation
                # group must CLOSE before the next group opens in the same
                # psum bank (zero-region constraint), so keep x/h pairs
                # adjacent: [mmx1, mmh1, mmx2, mmh2]. ---
                _tag(nc.tensor.matmul(ps[:, 0:BS], s1[0:KX, :], rx,
                                 start=True, stop=first, tile_position=(0, 0)),
                     "mmx1", s, j)
                if not first:
                    _tag(nc.tensor.matmul(ps[:, 0:BS], s1[64:128, :], h_prev[s],
                                     start=False, stop=True,
                                     tile_position=(64, 0)), "mmh1", s, j)
                _tag(nc.tensor.matmul(ps[:, BS:2 * BS], s2[0:KX, :], rx,
                                 start=True, stop=first, tile_position=(0, 0)),
                     "mmx2", s, j)
                if not first:
                    _tag(nc.tensor.matmul(ps[:, BS:2 * BS], s2[64:128, :],
                                     h_prev[s], start=False, stop=True,
                                     tile_position=(64, 0)), "mmh2", s, j)
                # --- ACT: this slot's gates tanh, then pending tanh-c ---
                g = gpool.tile([128, 2 * BS], BF16, tag="g", name=f"g{s}{j}")
                _tag(nc.scalar.activation(g[:], ps[:], TANH), "g", s, j)
                if pend is not None:
                    tcb = tpool.tile([128, BS], BF16, tag="tc",
                                     name=f"tcb{sp}{jp}")
                    _tag(nc.scalar.activation(tcb[64:128, :], c_prev[sp], TANH,
                                              scale=0.5), "tc", sp, jp)
                # --- DVE: this slot's u', W, then pending h' ---
                wu = tpool.tile([128, BS], BF16, tag="wu", name=f"wu{s}{j}")
                _tag(nc.vector.scalar_tensor_tensor(
                    wu[64:128, :], g[0:H, 0:BS], 1.0, g[0:H, BS:2 * BS],
                    op0=ADD, op1=MUL), "u", s, j)
                if pend is not None:
                    gp, hhp, offp = pend[3], pend[4], pend[5]
                    h_out = hhp[64:128, offp:offp + BS]
                    _tag(nc.vector.scalar_tensor_tensor(
                        h_out, gp[H:128, BS:2 * BS], 1.0, tcb[64:128, :],
                        op0=ADD, op1=MUL), "hp", sp, jp)
                    h_prev[sp] = h_out
                # W deferred one slot: keeps it outside the engine's ready-
                # bypass window when the pending h' becomes runnable.
                if pend_W:
                    pend_W.pop(0)()
                if c_prev[s] is None:
                    nc.vector.memset(wu[0:H, :], 0.0)
                else:
                    def _emit_W(s=s, j=j, wu=wu, g=g, cp=c_prev[s]):
                        _tag(nc.vector.scalar_tensor_tensor(
                            wu[0:H, :], g[H:128, 0:BS], 1.0, cp,
                            op0=ADD, op1=MUL), "W", s, j)
                    pend_W.append(_emit_W)
                if pend is not None:
                    cip = jp // TC
                    if jp == cip * TC + TC - 1 and sp == NS - 1:
                        nc.sync.dma_start(
                            hs_d[cip * TC:(cip + 1) * TC]
                            .rearrange("t h b -> h t b"),
                            hhs[cip][64:128, :]
                            .rearrange("h (t b) -> h t b", t=TC),
                        )
                return (s, j, wu, g, hh, off)

            def emit_final(pend):
                sp, jp, wup, gp, hhp, offp = pend
                cps = cspool.tile([128, BS], F32, tag="cps",
                                  name=f"cpsf{sp}{jp}")
                nc.tensor.matmul(cps[:], pmat[:], wup[:], start=True, stop=True,
                                 tile_position=(0, 0))
                c_prev[sp] = cps[64:128, :]
                tcb = tpool.tile([128, BS], BF16, tag="tc", name=f"tcf{sp}{jp}")
                nc.scalar.activation(tcb[64:128, :], c_prev[sp], TANH, scale=0.5)
                h_out = hhp[64:128, offp:offp + BS]
                nc.vector.scalar_tensor_tensor(
                    h_out, gp[H:128, BS:2 * BS], 1.0, tcb[64:128, :],
                    op0=ADD, op1=MUL)
                cip = jp // TC
                if jp == cip * TC + TC - 1 and sp == NS - 1:
                    nc.sync.dma_start(
                        hs_d[cip * TC:(cip + 1) * TC].rearrange("t h b -> h t b"),
                        hhs[cip][64:128, :].rearrange("h (t b) -> h t b", t=TC),
                    )

            pending = []
            for j in range(T):
                for s in range(NS):
                    pend = pending.pop(0) if len(pending) >= LAG else None
                    pending.append(emit_slot(s, j, pend))
            while pend_W:
                pend_W.pop(0)()
            for p in pending:
                emit_final(p)
    _split_waits(nc, max_waits=1)
    _cache["nc"] = nc
    return nc


def _prep_core_inputs(input_data, W_ih, W_hh, b_ih, b_hh):
    bias = (b_ih + b_hh).astype(np.float32)           # [256]
    W_ihT = W_ih.astype(np.float32).T.copy()          # [22, 256]
    W_hhT = W_hh.astype(np.float32).T.copy()          # [64, 256]
    # scale sigmoid-gate rows (i: 0:64, f: 64:128, o: 192:256) by 0.5 for
    # the uniform-tanh trick; g rows (128:192) stay unscaled
    scale = np.ones(256, np.float32) * 0.5
    scale[128:192] = 1.0
    W_ihT *= scale
    bias *= scale
    # W_hh consumes h' = 2h from the history strip -> extra 0.5
    W_hhT *= scale * 0.5

    def stationary(lo, hi):
        s = np.zeros((128, 128), np.float32)
        s[0:IN, :] = W_ihT[:, lo:hi]
        s[IN, :] = bias[lo:hi]
        s[64:128, :] = W_hhT[:, lo:hi]
        return s.astype(ml_dtypes.bfloat16)

    s1 = stationary(0, 128)
    s2 = stationary(128, 256)
    # c_psum[64+m] = 0.5*wu[m] + wu[64+m]  (wu rows 0:64 = W, 64:128 = u')
    pm = np.zeros((128, 128), np.float32)
    for m in range(64):
        pm[m, 64 + m] = 0.5
        pm[64 + m, 64 + m] = 1.0
    pm = pm.astype(ml_dtypes.bfloat16)

    x8 = input_data.reshape(N_CORES, B, T, IN)
    in_maps = []
    for c in range(N_CORES):
        xT = np.empty((T, KX, B), ml_dtypes.bfloat16)
        xT[:, 0:IN, :] = x8[c].transpose(1, 2, 0).astype(ml_dtypes.bfloat16)
        xT[:, IN, :] = 1.0
        in_maps.append({"xT": np.ascontiguousarray(xT), "S1": s1, "S2": s2,
                        "P": pm})
    return in_maps


def kernel(input_data, W_ih, W_hh, b_ih, b_hh):
    input_data = np.asarray(input_data, np.float32)
    W_ih = np.asarray(W_ih, np.float32)
    W_hh = np.asarray(W_hh, np.float32)
    b_ih = np.asarray(b_ih, np.float32)
    b_hh = np.asarray(b_hh, np.float32)

    nc = _build()
    in_maps = _prep_core_inputs(input_data, W_ih, W_hh, b_ih, b_hh)
    res = bass_utils.run_bass_kernel_spmd(nc, in_maps, core_ids=list(range(N_CORES)))
    _cache["last_results"] = res

    out = np.empty((B_FULL, T, H), np.float32)
    for c in range(N_CORES):
        hs = res.results[c]["hs"].astype(np.float32)  # [T, H, B] (holds 2h)
        out[c * B:(c + 1) * B] = hs.transpose(2, 0, 1)
    out *= 0.5
    return out


# revision 20
# speedup vs baseline: 1.2680x; 1.0109x over previous
"""Bass/Trainium2 LSTM encoder kernel.

Problem: nn_Encoder (LSTM): input [B=4096, T=512, IN=22], hidden H=64,
torch gate order i,f,g,o. Output: hidden states [B, T, H].

Sharding: data-parallel over batch across 8 NeuronCores (512 batch rows per
core, split into two software-pipelined streams of 256). Weights replicated.
The T=512 recurrence runs sequentially per core.

Per-core structure (feature-on-partition, batch in the free dim), bf16
datapath (PSUM accumulation stays fp32):
  - x host-transposed to xT [T, 23, B] bf16; row 22 is ones, so the bias
    rides the x-matmul (K=23).
  - Stationary S1/S2 [128, 128] bf16: rows 0:22 W_ihT gate-chunk, row 22
    combined bias, rows 64:128 W_hhT gate-chunk. Per stream-step, two
    row-tiled matmuls per chunk (x-part at array rows 0:23, h-part at rows
    64:128) accumulate one psum bank [128, 512] (chunk1 cols 0:256,
    chunk2 256:512).
  - Uniform-tanh trick: sigmoid-gate rows (i, f, o) of W and bias are
    pre-scaled 0.5 on the host (sigmoid(z) = 0.5 tanh(0.5 z) + 0.5), so ONE
    tanh activation over the whole psum produces all gates as bf16:
    G = [ti;tf|tg;to].
  - Scaled state C = 2c and history h' = 2h (host halves the output); all
    elementwise ops are bf16 scalar_tensor_tensor on DVE (2-byte dtype
    engages the DVE fast modes):
      u' = (ti+1)*tg
      W  = (tf+1)*C
      C  = 0.5 W + u'       (C persistent per stream, memset once)
      tc = tanh(0.5 C)      activation with input scale (ACT)
      h' = (to+1)*tc
  - Two batch streams of 256 are interleaved so the per-step dependency
    chain of one stream overlaps the other stream's engine work.
  - h' written into an SBUF history strip (base partition 64, aligned with
    the W_hh array rows), DMA'd out every TC steps as hs [T, H, B] bf16; the
    host transposes back to [B, T, H] and multiplies by 0.5.
Walrus in this container accepts at most ONE semaphore wait per instruction;
_split_waits post-processes Tile's output to satisfy that.
"""

import numpy as np
import ml_dtypes

import bass_rust
import concourse.bass as bass
import concourse.mybir as mybir
import concourse.tile as tile
import concourse.bass_utils as bass_utils

N_CORES = 8
B_FULL, T, IN, H = 4096, 512, 22, 64
B = B_FULL // N_CORES          # batch per core
NS = 4                         # software-pipelined streams
BS = B // NS                   # batch per stream
LAG = 2                        # quarter-slots between a head and its tail
KX = IN + 1                    # x rows + ones row
TC = 16                        # timesteps per DMA chunk
F32 = mybir.dt.float32
BF16 = mybir.dt.bfloat16

# tail(s, j-1) must be emitted before head(s, j): slot NS*(j-1)+s+LAG < NS*j+s
assert LAG < NS

_cache = {}
ROLES = {}      # instruction name -> (role, stream, step); for trace analysis


def _tag(inst, role, s, j):
    try:
        ROLES[inst.ins.name] = (role, s, j)
    except Exception:
        pass
    return inst


def _split_waits(nc, max_waits=1):
    """walrus here allows one sem-wait per instruction; split extras into
    preceding same-engine NOPs."""
    for f in nc.m.functions:
        for bb in f.blocks:
            insts = bb.instructions
            changed = False
            out = []
            for inst in insts:
                si = inst.sync_info
                if si is not None and si.on_wait and len(si.on_wait) > max_waits:
                    waits = list(si.on_wait)
                    head, rest = waits[:-max_waits], waits[-max_waits:]
                    for i in range(0, len(head), max_waits):
                        nop = mybir.InstNoOp(name=nc.get_next_instruction_name())
                        nop.engine = inst.engine
                        nop.sync_info = bass_rust.SyncInfo(
                            on_wait=head[i:i + max_waits], on_update=[])
                        out.append(nop)
                    inst.sync_info = bass_rust.SyncInfo(
                        on_wait=rest, on_update=list(si.on_update))
                    changed = True
                out.append(inst)
            if changed:
                cur = bb.instructions
                del cur[:]
                cur.extend(out)


def _build():
    if "nc" in _cache:
        return _cache["nc"]

    nc = bass.Bass("TRN2", target_bir_lowering=False, debug=False,
                   enable_asserts=False, num_devices=1)

    xT_d = nc.dram_tensor("xT", [T, 64, B], BF16, kind="ExternalInput").ap()
    h0_d = nc.dram_tensor("H0", [64, B], BF16, kind="ExternalInput").ap()
    s1_d = nc.dram_tensor("S1", [128, 128], BF16, kind="ExternalInput").ap()
    s2_d = nc.dram_tensor("S2", [128, 128], BF16, kind="ExternalInput").ap()
    p_d = nc.dram_tensor("P", [128, 128], BF16, kind="ExternalInput").ap()
    hs_d = nc.dram_tensor("hs", [T, H, B], BF16, kind="ExternalOutput").ap()

    TANH = mybir.ActivationFunctionType.Tanh
    ADD = mybir.AluOpType.add
    MUL = mybir.AluOpType.mult

    n_chunks = T // TC

    with tile.TileContext(nc) as tc:
        with (
            tc.tile_pool(name="const", bufs=1) as cpool,
            tc.tile_pool(name="xin", bufs=3) as xpool,
            tc.tile_pool(name="hh", bufs=2) as hpool,
            tc.tile_pool(name="gates", bufs=8) as gpool,
            tc.tile_pool(name="tmp", bufs=8) as tpool,
            tc.tile_pool(name="ps", bufs=4, space="PSUM") as pspool,
            tc.tile_pool(name="cs", bufs=4, space="PSUM") as cspool,
        ):
            s1 = cpool.tile([128, 128], BF16, tag="s1")
            s2 = cpool.tile([128, 128], BF16, tag="s2")
            pmat = cpool.tile([128, 128], BF16, tag="pmat")
            nc.sync.dma_start(s1[:], s1_d[:])
            nc.sync.dma_start(s2[:], s2_d[:])
            nc.sync.dma_start(pmat[:], p_d[:])

            # Software-pipelined schedule over (stream, step) slots.
            # At slot k (stream s = k % NS, step j = k // NS) we emit, in
            # program order per engine, the HEAD of (s, j) and the TAIL of
            # the slot-(k-LAG) stream-step (sp, jp):
            #   PE : 4 gate matmuls (s,j) ; cps (sp,jp)  [C' = 0.5W + u']
            #   ACT: tanh gates (s,j)     ; tanh-c (sp,jp)
            #   DVE: u',W (s,j)           ; h' (sp,jp)
            # The LAG matches the g->wu->cps latency so no engine ever
            # blocks in-order on a cross-engine round trip: every tail op's
            # inputs are ready by the time its engine reaches it.
            c_prev = [None] * NS     # psum C AP per stream (None -> zeros)
            pend_W = []              # deferred W-stst emitters
            xchs = {}
            # landing tile for the final step's h' (never fed back)
            xfin = hpool.tile([128, B], BF16, tag="xf", name="xfin")

            def ensure_chunk(ci):
                if ci not in xchs:
                    xch = xpool.tile([128, TC * B], BF16, tag="x",
                                     name=f"xch{ci}")
                    nc.sync.dma_start(
                        xch[0:64, :].rearrange("k (t b) -> k t b", t=TC),
                        xT_d[ci * TC:(ci + 1) * TC].rearrange("t k b -> k t b"),
                    )
                    if ci == 0:
                        # initial hidden state h(-1) = 0 into slot 0
                        nc.sync.dma_start(xch[64:128, 0:B], h0_d[:])
                    xchs[ci] = xch
                return xchs[ci]

            def emit_slot(s, j, pend):
                ci = j // TC
                xch = ensure_chunk(ci)
                off = (j - ci * TC) * B + s * BS
                rx = xch[:, off:off + BS]
                ps = pspool.tile([128, 2 * BS], F32, tag="ps", name=f"ps{s}{j}")
                # --- PE: pending tail's cps first (input ready since the
                # previous slot; must not queue behind anything) ---
                if pend is not None:
                    sp, jp, wup = pend[0], pend[1], pend[2]
                    cps = cspool.tile([128, BS], F32, tag="cps",
                                      name=f"cps{sp}{jp}")
                    _tag(nc.tensor.matmul(cps[:], pmat[:], wup[:], start=True,
                                     stop=True, tile_position=(0, 0)),
                         "cps", sp, jp)
                    c_prev[sp] = cps[64:128, :]
                # --- PE: merged [x|h] gate matmuls, K=128: rows 0:22 x,
                # row 22 ones, rows 23:64 zeros, rows 64:128 h(t-1) (written
                # into this step's column slot by the previous step's h'). ---
                _tag(nc.tensor.matmul(ps[:, 0:BS], s1[:], rx,
                                 start=True, stop=True, tile_position=(0, 0)),
                     "mmh1", s, j)
                _tag(nc.tensor.matmul(ps[:, BS:2 * BS], s2[:], rx,
                                 start=True, stop=True, tile_position=(0, 0)),
                     "mmh2", s, j)
                # --- ACT: this slot's gates tanh, then pending tanh-c ---
                g = gpool.tile([128, 2 * BS], BF16, tag="g", name=f"g{s}{j}")
                _tag(nc.scalar.activation(g[:], ps[:], TANH), "g", s, j)
                if pend is not None:
                    tcb = tpool.tile([128, BS], BF16, tag="tc",
                                     name=f"tcb{sp}{jp}")
                    _tag(nc.scalar.activation(tcb[64:128, :], c_prev[sp], TANH,
                                              scale=0.5), "tc", sp, jp)
                # --- DVE: this slot's u', W, then pending h' ---
                wu = tpool.tile([128, BS], BF16, tag="wu", name=f"wu{s}{j}")
                _tag(nc.vector.scalar_tensor_tensor(
                    wu[64:128, :], g[0:H, 0:BS], 1.0, g[0:H, BS:2 * BS],
                    op0=ADD, op1=MUL), "u", s, j)
                if pend is not None:
                    gp = pend[3]
                    tn = pend[1] + 1          # h' lands in step tn's slot
                    if tn < T:
                        cin = tn // TC
                        xn = ensure_chunk(cin)
                        offn = (tn - cin * TC) * B + sp * BS
                        h_out = xn[64:128, offn:offn + BS]
                    else:
                        h_out = xfin[64:128, sp * BS:(sp + 1) * BS]
                    _tag(nc.vector.scalar_tensor_tensor(
                        h_out, gp[H:128, BS:2 * BS], 1.0, tcb[64:128, :],
                        op0=ADD, op1=MUL), "hp", sp, jp)
                # W deferred one slot: keeps it outside the engine's ready-
                # bypass window when the pending h' becomes runnable.
                if pend_W:
                    pend_W.pop(0)()
                if c_prev[s] is None:
                    nc.vector.memset(wu[0:H, :], 0.0)
                else:
                    def _emit_W(s=s, j=j, wu=wu, g=g, cp=c_prev[s]):
                        _tag(nc.vector.scalar_tensor_tensor(
                            wu[0:H, :], g[H:128, 0:BS], 1.0, cp,
                            op0=ADD, op1=MUL), "W", s, j)
                    pend_W.append(_emit_W)
                if pend is not None and sp == NS - 1:
                    emit_out_dma(jp)
                return (s, j, wu, g)

            def emit_out_dma(jp):
                # all h' for steps <= jp written; chunk ci is complete once
                # its last slot (holding h of step ci*TC+TC-2) is filled
                tn = jp + 1
                if tn % TC != 0:
                    return
                ci = tn // TC - 1
                xc = xchs[ci]
                if ci == 0:
                    nc.sync.dma_start(
                        hs_d[0:TC - 1].rearrange("t h b -> h t b"),
                        xc[64:128, B:].rearrange("h (t b) -> h t b", t=TC - 1),
                    )
                else:
                    nc.sync.dma_start(
                        hs_d[ci * TC - 1:ci * TC + TC - 1]
                        .rearrange("t h b -> h t b"),
                        xc[64:128, :].rearrange("h (t b) -> h t b", t=TC),
                    )

            def emit_final(pend):
                sp, jp, wup, gp = pend[0], pend[1], pend[2], pend[3]
                cps = cspool.tile([128, BS], F32, tag="cps",
                                  name=f"cpsf{sp}{jp}")
                nc.tensor.matmul(cps[:], pmat[:], wup[:], start=True, stop=True,
                                 tile_position=(0, 0))
                c_prev[sp] = cps[64:128, :]
                tcb = tpool.tile([128, BS], BF16, tag="tc", name=f"tcf{sp}{jp}")
                nc.scalar.activation(tcb[64:128, :], c_prev[sp], TANH, scale=0.5)
                h_out = xfin[64:128, sp * BS:(sp + 1) * BS]
                nc.vector.scalar_tensor_tensor(
                    h_out, gp[H:128, BS:2 * BS], 1.0, tcb[64:128, :],
                    op0=ADD, op1=MUL)
                if sp == NS - 1:
                    emit_out_dma(jp)
                    nc.sync.dma_start(
                        hs_d[T - 1:T].rearrange("t h b -> h t b"),
                        xfin[64:128, :].rearrange("h (t b) -> h t b", t=1),
                    )

            pending = []
            for j in range(T):
                for s in range(NS):
                    pend = pending.pop(0) if len(pending) >= LAG else None
                    pending.append(emit_slot(s, j, pend))
            while pend_W:
                pend_W.pop(0)()
            for p in pending:
                emit_final(p)
    _split_waits(nc, max_waits=1)
    _cache["nc"] = nc
    return nc


def _prep_core_inputs(input_data, W_ih, W_hh, b_ih, b_hh):
    bias = (b_ih + b_hh).astype(np.float32)           # [256]
    W_ihT = W_ih.astype(np.float32).T.copy()          # [22, 256]
    W_hhT = W_hh.astype(np.float32).T.copy()          # [64, 256]
    # scale sigmoid-gate rows (i: 0:64, f: 64:128, o: 192:256) by 0.5 for
    # the uniform-tanh trick; g rows (128:192) stay unscaled
    scale = np.ones(256, np.float32) * 0.5
    scale[128:192] = 1.0
    W_ihT *= scale
    bias *= scale
    # W_hh consumes h' = 2h from the history strip -> extra 0.5
    W_hhT *= scale * 0.5

    def stationary(lo, hi):
        s = np.zeros((128, 128), np.float32)
        s[0:IN, :] = W_ihT[:, lo:hi]
        s[IN, :] = bias[lo:hi]
        s[64:128, :] = W_hhT[:, lo:hi]
        return s.astype(ml_dtypes.bfloat16)

    s1 = stationary(0, 128)
    s2 = stationary(128, 256)
    # c_psum[64+m] = 0.5*wu[m] + wu[64+m]  (wu rows 0:64 = W, 64:128 = u')
    pm = np.zeros((128, 128), np.float32)
    for m in range(64):
        pm[m, 64 + m] = 0.5
        pm[64 + m, 64 + m] = 1.0
    pm = pm.astype(ml_dtypes.bfloat16)

    x8 = input_data.reshape(N_CORES, B, T, IN)
    h0 = np.zeros((64, B), ml_dtypes.bfloat16)
    in_maps = []
    for c in range(N_CORES):
        xT = np.zeros((T, 64, B), ml_dtypes.bfloat16)
        xT[:, 0:IN, :] = x8[c].transpose(1, 2, 0).astype(ml_dtypes.bfloat16)
        xT[:, IN, :] = 1.0
        in_maps.append({"xT": np.ascontiguousarray(xT), "S1": s1, "S2": s2,
                        "P": pm, "H0": h0})
    return in_maps


def kernel(input_data, W_ih, W_hh, b_ih, b_hh):
    input_data = np.asarray(input_data, np.float32)
    W_ih = np.asarray(W_ih, np.float32)
    W_hh = np.asarray(W_hh, np.float32)
    b_ih = np.asarray(b_ih, np.float32)
    b_hh = np.asarray(b_hh, np.float32)

    nc = _build()
    in_maps = _prep_core_inputs(input_data, W_ih, W_hh, b_ih, b_hh)
    res = bass_utils.run_bass_kernel_spmd(nc, in_maps, core_ids=list(range(N_CORES)))
    _cache["last_results"] = res

    out = np.empty((B_FULL, T, H), np.float32)
    for c in range(N_CORES):
        hs = res.results[c]["hs"].astype(np.float32)  # [T, H, B] (holds 2h)
        out[c * B:(c + 1) * B] = hs.transpose(2, 0, 1)
    out *= 0.5
    return out
